# revision 9
# baseline (speedup 1.0000x reference)
"""Block-diagonal rotation (COB) kernel for Trainium2, 8 NeuronCores.

Computes out[..., block_i] = x[..., block_i] @ W_i.T for 8 square blocks of
sizes [512, 1024, 256, 768, 384, 640, 128, 384] (features sum to 4096),
x shape (4, 2048, 4096) fp32.

Strategy (v3 = v1 pipeline structure in bf16):
  - Pure data-parallel over rows: 8192 rows are split 8 ways (1024/core).
  - All tensors are converted to bf16 on the host (harness tolerance is
    2e-2; bf16 end-to-end lands ~4e-3), halving HBM traffic vs fp32:
    21.1 MiB/core (x-in 8 + w 5.1 + out 8) ~= 62 us at ~358 GB/s.
  - x tiles [128, 4096] are DMA'd naturally (rows on partitions),
    transposed 128x128 on the TensorEngine (bf16 transpose = 1 cycle/row
    vs 1.5 for f32r), PSUM->SBUF copied by the VectorEngine, then used as
    the stationary operand of bf16 matmuls against resident weight tiles.
  - PSUM fp32 accumulations are downcast-copied to bf16 staging tiles
    (alternating DVE/ACT) and DMA'd out as 0.5 MiB stores.
  - Software pipelining as v1: transposes for row-tile r+2 interleave
    into row-tile r's matmuls; weight chunks alternate between the two
    HWDGE rings (scalar + sync) to halve preload latency.
"""

import numpy as np
import ml_dtypes

import concourse.bacc as bacc
import concourse.mybir as mybir
from concourse.tile import TileContext
from concourse.bass_utils import run_bass_kernel_spmd

SIZES = [512, 1024, 256, 768, 384, 640, 128, 384]
OFFS = np.cumsum([0] + SIZES)
N_CORES = 8
ROWS_TOTAL = 4 * 2048
ROWS_PER_CORE = ROWS_TOTAL // N_CORES  # 1024
D = 4096
P = 128
R_TILES = ROWS_PER_CORE // P  # 8

# e-slices per block: PSUM bank holds 512 fp32 per partition
E_SLICES = {
    512: [512], 1024: [512, 512], 256: [256], 768: [512, 256],
    384: [384], 640: [384, 256], 128: [128],
}

BF16 = mybir.dt.bfloat16
F32 = mybir.dt.float32

_cache = {}


def build_nc():
    if "nc" in _cache:
        return _cache["nc"]
    nc = bacc.Bacc()
    x_d = nc.declare_dram_parameter("x", [ROWS_PER_CORE, D], BF16, isOutput=False)
    w_d = [
        nc.declare_dram_parameter(f"w{i}", [s, s], BF16, isOutput=False)
        for i, s in enumerate(SIZES)
    ]
    id_d = nc.declare_dram_parameter("ident", [P, P], BF16, isOutput=False)
    out_d = nc.declare_dram_parameter("out", [ROWS_PER_CORE, D], BF16, isOutput=True)

    x_v = x_d.rearrange("(r p) d -> r p d", p=P)
    out_v = out_d.rearrange("(r p) d -> r p d", p=P)

    with TileContext(nc) as tc:
        with (
            tc.tile_pool(name="wres", bufs=1) as wres,
            tc.tile_pool(name="xnat", bufs=2) as xnat_p,
            tc.tile_pool(name="xt", bufs=3) as xt_p,
            tc.tile_pool(name="osb", bufs=2) as osb_p,
            tc.tile_pool(name="idp", bufs=1) as idp,
            tc.tile_pool(name="tp", bufs=2, space="PSUM") as tp_p,
            tc.tile_pool(name="mm", bufs=4, space="PSUM") as mm_p,
        ):
            # identity (bf16) for PE transpose — DMA'd from DRAM so the
            # first transpose doesn't wait on DVE table loads / iota setup
            ident = idp.tile([P, P], BF16, tag="idb")
            nc.scalar.dma_start(out=ident[:], in_=id_d[:, :])

            # resident weights: per block, per k-tile: [128, s] bf16.
            # Even-numbered chunks stream on the scalar HWDGE ring now;
            # odd-numbered chunks go on the sync ring, queued after the
            # prologue x tiles (deferred emission below).
            wt = []
            w_sync_dmas = {i: [] for i in range(len(SIZES))}
            ci = 0
            for i, s in enumerate(SIZES):
                w_v = w_d[i].rearrange("(k p) e -> k p e", p=P)
                ks = []
                for k in range(s // P):
                    t = wres.tile([P, s], BF16, tag=f"w{i}_{k}")
                    if ci % 2 == 0:
                        nc.scalar.dma_start(out=t[:], in_=w_v[k])
                    else:
                        w_sync_dmas[i].append((t, w_v[k]))
                    ks.append(t)
                    ci += 1
                wt.append(ks)

            # Software pipeline over row-tiles (demand-driven transpose pump,
            # sliding window of up to 2 row-tiles of transposed x).
            xnat = {}
            xts_all = {}

            def issue_x_dma(r, quarters=False):
                lo = xnat_p.tile([P, D // 2], BF16, tag="xnl")
                hi = xnat_p.tile([P, D // 2], BF16, tag="xnh")
                if quarters:
                    q = D // 4
                    nc.sync.dma_start(out=lo[:, :q], in_=x_v[r][:, :q])
                    nc.sync.dma_start(out=lo[:, q:], in_=x_v[r][:, q:2 * q])
                    nc.sync.dma_start(out=hi[:, :q], in_=x_v[r][:, 2 * q:3 * q])
                    nc.sync.dma_start(out=hi[:, q:], in_=x_v[r][:, 3 * q:])
                else:
                    nc.sync.dma_start(out=lo[:], in_=x_v[r][:, :D // 2])
                    nc.sync.dma_start(out=hi[:], in_=x_v[r][:, D // 2:])
                xnat[r] = (lo, hi)

            def transpose_group(r, j):
                # transposes d-tiles 4j..4j+3 of row-tile r into xt tile j
                lo, hi = xnat[r]
                src = lo if j < 4 else hi
                base = P * 4 * j - (0 if j < 4 else D // 2)
                ps = tp_p.tile([P, 4 * P], BF16, tag="tp")
                for i in range(4):
                    nc.tensor.transpose(
                        ps[:, P * i:P * (i + 1)],
                        src[:, base + P * i:base + P * (i + 1)],
                        ident[:],
                    )
                xt = xt_p.tile([P, 4 * P], BF16, tag=f"xt{j}")
                nc.vector.tensor_copy(xt[:], ps[:])
                xts_all.setdefault(r, {})[j] = xt

            tp_queue = [(r, j) for r in range(R_TILES) for j in range(8)]
            state = {"cursor": 0}

            def pump_to(idx):
                idx = min(idx, len(tp_queue))
                while state["cursor"] < idx:
                    r_, j_ = tp_queue[state["cursor"]]
                    transpose_group(r_, j_)
                    state["cursor"] += 1

            # j-group needed to cover all d-tiles of block b
            J_HI = [(int(OFFS[b + 1]) - 1) // 512 for b in range(len(SIZES))]

            issue_x_dma(0, quarters=True)
            for i in (0, 1):
                for t, src in w_sync_dmas[i]:
                    nc.sync.dma_start(out=t[:], in_=src)
            issue_x_dma(1)
            for i in range(2, len(SIZES)):
                for t, src in w_sync_dmas[i]:
                    nc.sync.dma_start(out=t[:], in_=src)

            for r in range(R_TILES):
                last = r == R_TILES - 1
                if r + 2 < R_TILES:
                    issue_x_dma(r + 2)
                o_t = osb_p.tile([P, D], BF16, tag="os")
                for b, s in enumerate(SIZES):
                    pump_to(r * 8 + J_HI[b] + 1)
                    xts = xts_all[r]
                    d0 = int(OFFS[b]) // P
                    kt = s // P
                    n0 = 0
                    for nw in E_SLICES[s]:
                        ps = mm_p.tile([P, nw], F32, tag="mm", name="mmps")
                        for k in range(kt):
                            g = d0 + k
                            lhsT = xts[g // 4][:, P * (g % 4):P * (g % 4 + 1)]
                            nc.tensor.matmul(
                                ps[:], lhsT, wt[b][k][:, n0:n0 + nw],
                                start=(k == 0), stop=(k == kt - 1),
                            )
                        dst = o_t[:, int(OFFS[b]) + n0:int(OFFS[b]) + n0 + nw]
                        if (r + b) % 2 == 0:
                            nc.scalar.copy(dst, ps[:])
                        else:
                            nc.vector.tensor_copy(dst, ps[:])
                        if last:
                            # stream the final row-tile out per-slice so the
                            # tail store overlaps the remaining compute
                            c0 = int(OFFS[b]) + n0
                            nc.sync.dma_start(out=out_v[r][:, c0:c0 + nw],
                                              in_=o_t[:, c0:c0 + nw])
                        n0 += nw
                    pump_to(r * 8 + b + 17)
                del xts_all[r]
                if not last:
                    nc.sync.dma_start(out=out_v[r][:, :D // 2], in_=o_t[:, :D // 2])
                    nc.sync.dma_start(out=out_v[r][:, D // 2:], in_=o_t[:, D // 2:])

    nc.finalize()
    _cache["nc"] = nc
    return nc


def build_in_maps(x, w0, w1, w2, w3, w4, w5, w6, w7):
    x = np.asarray(x, dtype=np.float32).reshape(ROWS_TOTAL, D)
    xb = x.astype(ml_dtypes.bfloat16)
    ws = [w0, w1, w2, w3, w4, w5, w6, w7]
    wts = [
        np.ascontiguousarray(np.asarray(w, dtype=np.float32).T).astype(
            ml_dtypes.bfloat16
        )
        for w in ws
    ]
    ident = np.eye(P, dtype=np.float32).astype(ml_dtypes.bfloat16)
    in_maps = []
    for c in range(N_CORES):
        m = {"x": xb[c * ROWS_PER_CORE:(c + 1) * ROWS_PER_CORE], "ident": ident}
        for i, wtb in enumerate(wts):
            m[f"w{i}"] = wtb
        in_maps.append(m)
    return in_maps


def kernel(x, w0, w1, w2, w3, w4, w5, w6, w7):
    nc = build_nc()
    in_maps = build_in_maps(x, w0, w1, w2, w3, w4, w5, w6, w7)
    res = run_bass_kernel_spmd(nc, in_maps, list(range(N_CORES)))
    out = np.concatenate([r["out"] for r in res.results], axis=0)
    return out.reshape(4, 2048, D).astype(np.float32)
